# revision 2
# baseline (speedup 1.0000x reference)
"""GroupedQueryAttention (B=1, S=2048, D=4096, 32 Q / 8 KV heads) on 8 TRN2 cores.

Fused single-scope pipeline: QKV projection, RoPE, causal attention and
out-projection partials share one pool set (8 PSUM banks total), so the
Tile scheduler can fill attention dependency stalls with projection /
out-proj matmuls.  ReduceScatter is row-chunked: chunk b ([S/RSC, D]) is
issued as soon as its out-proj rows are staged, overlapping the collective
with remaining compute.  Host reassembles the row-interleaved RS outputs.

Per core (one KV group = 4 Q heads + 1 KV head):
  - QKV projection as 6 sequential PSUM chains (q0..q3, k, v), mc-outer so
    each weight/x chunk is consumed right after its DMA lands; RoPE of one
    chain drains on DVE while the next chains run (no PSUM WAR stalls)
  - RoPE on q and k via DVE (1/sqrt(HD) split sqrt-wise onto both tables)
  - causal flash attention in scores^T orientation; diagonal tiles trimmed;
    causal mask folded into the QK PSUM via an identity matmul adding -3e38;
    softmax denominator accumulated per-tile on PE (ones-matmul), so the
    QK->exp->AV chain has no DVE hop
  - out-proj partial ep-units (8 matmuls vs own 512 Wo rows) interleaved
    into the attention t-tile loop, staged as [128,4096] 1MB DMAs on the
    ACT HWDGE ring; 8 row-chunked ReduceScatters issued as rows complete,
    overlapping the collective with remaining compute
"""
import numpy as np
import ml_dtypes

from concourse import bass, bacc, tile, mybir
from concourse.bass_utils import run_bass_kernel_spmd

BF16 = ml_dtypes.bfloat16
F32 = np.float32

D = 4096          # model dim
S = 2048          # sequence
NH = 32           # query heads
NG = 8            # kv heads == n cores
HD = 128          # head dim
G = NH // NG      # 4 query heads per group/core
KV = NG * HD      # 1024
BASE = 50000.0
SCALE = 1.0 / np.sqrt(HD)
N_CORES = 8
SC = S // 512     # 4 s-chunks of 512
MC = D // 128     # 32 contraction chunks
HALF = MC // 2    # x chunk split for SBUF
SSLICE = S // N_CORES  # 256 rows of final output per core
RSC = 8           # reduce-scatter row chunks
RSROWS = S // RSC         # 512 rows per RS chunk
RSOUT = RSROWS // N_CORES  # 64 rows per core per RS chunk
STS_PER_RS = 16 // RSC     # st row-blocks per RS chunk

_CACHE = {}


def _build(reps: int = 1, sim: bool = False, coll: str = "rs",
           nodma: bool = False, trim: bool = True, rsc: int = RSC,
           hasbias: bool = True, noattn: bool = False, ilv: bool = True):
    if sim:
        coll = "copy"
    sts_per = 16 // rsc
    rsrows = S // rsc
    rsout = rsrows // N_CORES

    def dma(dst, src, **kwargs):
        if not nodma:
            nc.sync.dma_start(dst, src, **kwargs)
        else:
            # timing probe: touch the tile with a tiny transfer so Tile
            # sees it written; data is garbage but schedule shape holds
            nc.sync.dma_start(dst[:, 0:64], src[:, 0:64], **kwargs)

    def dma_store(*args, **kwargs):
        # stores ride the ACT HWDGE ring so they never queue ahead of loads
        nc.scalar.dma_start(*args, **kwargs)
    f32 = mybir.dt.float32
    bf16 = mybir.dt.bfloat16

    nc = bacc.Bacc("TRN2", target_bir_lowering=False, debug=False,
                   num_devices=N_CORES)

    # ---- I/O ----
    xt_d = nc.dram_tensor("xt", [128, SC * MC * 512], bf16, kind="ExternalInput")
    # block-major qkv weights: [128, (blk, mc, 128)], blk = q0..q3, k, v
    wqkv_d = nc.dram_tensor("wqkv", [128, 6 * MC * 128], bf16,
                            kind="ExternalInput")
    wo_d = nc.dram_tensor("wo", [128, G * 8 * 512], bf16, kind="ExternalInput")
    cos_d = nc.dram_tensor("cos", [128, S], bf16, kind="ExternalInput")
    sin_d = nc.dram_tensor("sin", [128, S], bf16, kind="ExternalInput")
    bqw_d = nc.dram_tensor("bqw", [1, 6 * 128], bf16, kind="ExternalInput")
    ones_d = nc.dram_tensor("ones", [1, 512], bf16, kind="ExternalInput")
    mask_d = nc.dram_tensor("mask", [128, 4 * 512], bf16, kind="ExternalInput")
    ident_d = nc.dram_tensor("ident", [128, 128], bf16, kind="ExternalInput")
    onem_d = nc.dram_tensor("onem", [128, 128], bf16, kind="ExternalInput")
    out_d = nc.dram_tensor("out", [SSLICE, D], bf16, kind="ExternalOutput")

    Ident = mybir.ActivationFunctionType.Identity
    CopyF = mybir.ActivationFunctionType.Copy
    Exp = mybir.ActivationFunctionType.Exp
    mult = mybir.AluOpType.mult

    from contextlib import ExitStack
    with tile.TileContext(nc) as tc:
        with ExitStack() as stack:
            ent = stack.enter_context
            cp = ent(tc.tile_pool(name="const", bufs=1))
            wp = ent(tc.tile_pool(name="wts", bufs=1))
            pp = ent(tc.tile_pool(name="pers", bufs=1))
            qtp = ent(tc.tile_pool(name="qt", bufs=2))
            atp = ent(tc.tile_pool(name="at", bufs=2))
            xsp = ent(tc.tile_pool(name="xs", bufs=6))
            vtp = ent(tc.tile_pool(name="vt", bufs=2))
            rtp = ent(tc.tile_pool(name="rt", bufs=2))
            esp = ent(tc.tile_pool(name="es", bufs=4))
            recp = ent(tc.tile_pool(name="rc", bufs=2))
            osp = ent(tc.tile_pool(name="os", bufs=2))
            qkvp = ent(tc.tile_pool(name="qkv_ps", bufs=2, space="PSUM"))
            qkp = ent(tc.tile_pool(name="qk_ps", bufs=1, space="PSUM"))
            avp = ent(tc.tile_pool(name="av_ps", bufs=1, space="PSUM"))
            bpp = ent(tc.tile_pool(name="bp_ps", bufs=1, space="PSUM"))
            pop = ent(tc.tile_pool(name="po_ps", bufs=2, space="PSUM"))
            dramp = ent(tc.tile_pool(name="dram", bufs=1, space="DRAM"))
            # constants: loaded once
            bqw = cp.tile([1, 6 * 128], bf16)
            nc.sync.dma_start(bqw[:], bqw_d[:])
            onesr = cp.tile([1, 512], bf16); nc.sync.dma_start(onesr[:], ones_d[:])
            mask = cp.tile([128, 4 * 512], bf16); nc.sync.dma_start(mask[:], mask_d[:])
            ident = cp.tile([128, 128], bf16); nc.sync.dma_start(ident[:], ident_d[:])
            onem = cp.tile([128, 128], bf16); nc.sync.dma_start(onem[:], onem_d[:])
            cos = cp.tile([128, S], bf16); nc.sync.dma_start(cos[:], cos_d[:])
            sin = cp.tile([128, S], bf16); nc.sync.dma_start(sin[:], sin_d[:])

            for _rep in range(reps):
                # per-rep weight tiles (ring reuse serializes reps naturally)
                wqkv = wp.tile([128, 6 * MC * 128], bf16, name="wqkv")
                wo = wp.tile([128, G * 8 * 512], bf16, name="wo")
                kT = pp.tile([128, S], bf16, name="kT")
                vN = pp.tile([128, S], bf16, name="vN")

                # rep-start loads in consumption order (SP HWDGE ring FIFO)
                xh = {}
                QTR = MC // 4

                def load_xq(sc, i):
                    xh[(sc, i)] = xsp.tile([128, QTR * 512], bf16, name="xs")
                    dma(xh[(sc, i)][:],
                        xt_d[:, (sc * MC + i * QTR) * 512:
                             (sc * MC + (i + 1) * QTR) * 512])

                def load_x(sc):
                    for i in range(4):
                        load_xq(sc, i)

                # interleave x quarters with per-chain weight blocks
                for i in range(4):
                    load_xq(0, i)
                    dma(wqkv[:, i * 4096:(i + 1) * 4096],
                        wqkv_d[:, i * 4096:(i + 1) * 4096])
                for j in range(4, 6):
                    dma(wqkv[:, j * 4096:(j + 1) * 4096],
                        wqkv_d[:, j * 4096:(j + 1) * 4096])

                attnT = {}   # chunk -> tile handle

                def rope(dst, ps, sc):
                    # dst = ps*cos + swap64(ps)*sin_signed  (tables pre-scaled)
                    cs = slice(sc * 512, (sc + 1) * 512)
                    t1 = rtp.tile([128, 512], bf16, name="rt")
                    nc.vector.tensor_tensor(t1[:], ps[:], cos[:, cs], mult)
                    t2 = rtp.tile([128, 512], bf16, name="rt")
                    nc.vector.tensor_tensor(t2[0:64, :], ps[64:128, :],
                                            sin[0:64, cs], mult)
                    nc.vector.tensor_tensor(t2[64:128, :], ps[0:64, :],
                                            sin[64:128, cs], mult)
                    nc.vector.tensor_add(dst, t1[:], t2[:])

                def qkv_chunk(sc):
                    # 6 sequential chains (q0..q3, k, v): rope of chain j
                    # drains while chains j+1, j+2 run -> no PSUM WAR stall
                    qTc = qtp.tile([128, G * 512], bf16, name="qT")
                    cs = slice(sc * 512, (sc + 1) * 512)
                    for blk in range(6):
                        ps = qkvp.tile([128, 512], f32, name="qkv")
                        if hasbias:
                            nc.tensor.matmul(ps[:],
                                             bqw[:, blk * 128:(blk + 1) * 128],
                                             onesr[:], start=True, stop=False)
                        for mc in range(MC):
                            xsl = xh[(sc, mc // QTR)][
                                :, (mc % QTR) * 512:(mc % QTR) * 512 + 512]
                            lhs = wqkv[:, (blk * MC + mc) * 128:
                                       (blk * MC + mc) * 128 + 128]
                            nc.tensor.matmul(ps[:], lhs, xsl,
                                             start=(mc == 0 and not hasbias),
                                             stop=(mc == MC - 1))
                        if blk < 4:
                            rope(qTc[:, blk * 512:blk * 512 + 512], ps, sc)
                        elif blk == 4:
                            rope(kT[:, cs], ps, sc)
                        else:
                            vTt = vtp.tile([128, 512], bf16, name="vt")
                            nc.vector.tensor_copy(vTt[:], ps[:])
                            pst = pop.tile([128, 512], bf16, name="po")
                            for i in range(4):
                                nc.tensor.transpose(
                                    pst[:, i * 128:(i + 1) * 128],
                                    vTt[:, i * 128:(i + 1) * 128], ident[:])
                            nc.vector.tensor_copy(vN[:, cs], pst[:])
                    return qTc

                def piece_ep(pc, st, ep, osb):
                    # one out-proj ep unit: 8 matmuls + 2 copies; stages the
                    # finished [128, D] row-block after its last ep
                    r = st - 4 * pc
                    aT = attnT[pc]
                    e0, e1 = 2 * ep, 2 * ep + 1
                    poA = pop.tile([128, 512], f32, name="po")
                    poB = pop.tile([128, 512], f32, name="po")
                    for h in range(G):
                        lw = aT[:, h * 512 + r * 128:h * 512 + r * 128 + 128]
                        nc.tensor.matmul(
                            poA[:], lw,
                            wo[:, (h * 8 + e0) * 512:(h * 8 + e0) * 512 + 512],
                            start=(h == 0), stop=(h == G - 1))
                        nc.tensor.matmul(
                            poB[:], lw,
                            wo[:, (h * 8 + e1) * 512:(h * 8 + e1) * 512 + 512],
                            start=(h == 0), stop=(h == G - 1))
                    # keep ACT exp-only: both copies on DVE
                    nc.vector.tensor_copy(osb[:, e0 * 512:(e0 + 1) * 512],
                                          poA[:])
                    nc.vector.tensor_copy(osb[:, e1 * 512:(e1 + 1) * 512],
                                          poB[:])
                    if ep == 3:
                        if coll != "noout" and not nodma:
                            b = st // sts_per
                            rr = st % sts_per
                            dma_store(rs_in[b][rr * 128:(rr + 1) * 128, :],
                                      osb[:])
                        if (st + 1) % sts_per == 0:
                            issue_rs(st // sts_per)

                pending = []

                def queue_piece(pc, st):
                    osb = osp.tile([128, D], bf16, name="osb")
                    for ep in range(4):
                        pending.append((piece_ep, pc, st, ep, osb))

                def drain_piece(n=None):
                    k = len(pending) if n is None else min(n, len(pending))
                    for _ in range(k):
                        f, *a = pending.pop(0)
                        f(*a)

                def issue_rs(b):
                    if coll == "noout":
                        return
                    if coll == "copy":
                        dma_store(out_d[b * rsout:(b + 1) * rsout, :],
                                  rs_in[b][0:rsout, :])
                        return
                    ro = dramp.tile([rsout, D], bf16, name=f"rs_out{b}", bufs=2)
                    nc.gpsimd.collective_compute(
                        "ReduceScatter", mybir.AluOpType.add,
                        replica_groups=[list(range(N_CORES))],
                        ins=[rs_in[b].opt()], outs=[ro.opt()])
                    dma_store(out_d[b * rsout:(b + 1) * rsout, :], ro[:])

                def attn_head(sc, h, qTc):
                    # chain per t-tile: QK (+mask-add via identity matmul on
                    # diagonal tiles) -> exp (ACT) -> AV + denominator, both
                    # accumulated on PE.  No DVE op anywhere in the chain.
                    hs = h * 512
                    ntt = 4 * (sc + 1)
                    av = avp.tile([128, 512], f32, name="av")
                    bps = bpp.tile([128, 512], f32, name="bp")

                    def emit_qk(tb):
                        qk = qkp.tile([128, 1024], f32, name="qk")
                        q0s = []
                        for q2 in range(2):
                            tt = 2 * tb + q2
                            r = tt - 4 * sc
                            q0 = max(r, 0) * 128 if trim else 0
                            q0s.append(q0)
                            sl = slice(q2 * 512 + q0, (q2 + 1) * 512)
                            nc.tensor.matmul(
                                qk[:, sl],
                                kT[:, tt * 128:(tt + 1) * 128],
                                qTc[:, hs + q0:hs + 512],
                                start=True, stop=(r < 0))
                            if r >= 0:
                                # fold causal mask in: qk += I.T @ (-3e38
                                # staircase), only the 128-wide diagonal window
                                nc.tensor.matmul(
                                    qk[:, q2 * 512 + q0:q2 * 512 + q0 + 128],
                                    ident[:],
                                    mask[:, r * 512 + q0:r * 512 + q0 + 128],
                                    start=False, stop=True)
                        es = esp.tile([128, 1024], bf16, name="es")
                        if q0s[1] > 0:
                            for q2 in range(2):
                                q0 = q0s[q2]
                                nc.scalar.activation(
                                    es[:, q2 * 512 + q0:(q2 + 1) * 512],
                                    qk[:, q2 * 512 + q0:(q2 + 1) * 512], Exp)
                        else:
                            nc.scalar.activation(es[:], qk[:], Exp)
                        return es, q0s

                    def emit_av(tb, es, q0s):
                        for q2 in range(2):
                            tt = 2 * tb + q2
                            q0 = q0s[q2]
                            esl = es[:, q2 * 512 + q0:(q2 + 1) * 512]
                            nc.tensor.matmul(
                                av[:, q0:512], vN[:, tt * 128:(tt + 1) * 128],
                                esl, start=(tt == 0), stop=(tt == ntt - 1))
                            nc.tensor.matmul(
                                bps[:, q0:512], onem[:], esl,
                                start=(tt == 0), stop=(tt == ntt - 1))

                    prev = None
                    for tb in range(ntt // 2):
                        cur = (tb, *emit_qk(tb))
                        if prev is not None:
                            emit_av(*prev)
                        if ilv:
                            drain_piece(1)
                        prev = cur
                    emit_av(*prev)
                    recb = recp.tile([128, 512], f32, name="rec")
                    nc.vector.reciprocal(recb[:], bps[:])
                    nc.vector.tensor_tensor(attnT[sc][:, hs:hs + 512], av[:],
                                            recb[:], mult)

                if coll != "noout":
                    rs_in = [dramp.tile([rsrows, D], bf16, name=f"rs_in{b}",
                                        bufs=2) for b in range(rsc)]

                for sc in range(SC):
                    qTc = qkv_chunk(sc)
                    attnT[sc] = atp.tile([128, G * 512], bf16, name="attnT")
                    # prefetch next x chunk; wo after first chunk's x
                    if sc < SC - 1:
                        load_x(sc + 1)
                    if sc == 0 and not noattn:
                        for j in range(4):
                            dma(wo[:, j * 4096:(j + 1) * 4096],
                                wo_d[:, j * 4096:(j + 1) * 4096])
                    if noattn:
                        continue
                    for h in range(G):
                        attn_head(sc, h, qTc)
                        if sc >= 1:
                            queue_piece(sc - 1, 4 * (sc - 1) + h)
                            if not ilv:
                                drain_piece()
                    drain_piece()
                if noattn:
                    # consume qT/attnT rings so Tile sees them written+read
                    for sc in range(SC):
                        nc.vector.tensor_copy(attnT[sc][:, 0:512],
                                              kT[:, 0:512])
                    continue
                for st in range(12, 16):
                    queue_piece(3, st)
                drain_piece()
    nc.compile()
    return nc


def _prep_inputs(x, Wqkv, bqkv, Wo, bo):
    """Host-side shard prep. Returns in_maps for the 8 cores."""
    x0T = np.ascontiguousarray(np.asarray(x, F32)[0].T)          # [D, S]
    xt_t = np.ascontiguousarray(
        x0T.reshape(MC, 128, SC, 512).transpose(1, 2, 0, 3)
    ).reshape(128, SC * MC * 512).astype(BF16)

    Wqkv = np.asarray(Wqkv, F32)
    Wo = np.asarray(Wo, F32)
    bqkv = np.asarray(bqkv, F32)

    # rope tables (transposed [HD, S]); 1/sqrt(HD) split sqrt-wise onto q & k
    inv_freq = 1.0 / (BASE ** (np.arange(0, HD, 2, dtype=np.float64) / HD))
    t = np.arange(S, dtype=np.float64)
    freqs = np.outer(t, inv_freq)                                # [S, 64]
    emb = np.concatenate([freqs, freqs], axis=1)                 # [S, HD]
    s4 = SCALE ** 0.5
    cosT = (np.cos(emb).T * s4).astype(F32)                      # [HD, S]
    sinT = (np.sin(emb).T * s4).astype(F32)
    sin_signed = np.concatenate([-sinT[:64], sinT[64:]], axis=0)
    cos_h = np.ascontiguousarray(cosT).astype(BF16)
    sin_h = np.ascontiguousarray(sin_signed).astype(BF16)

    # additive causal masks for the 4 diagonal t-tiles: 0 where visible,
    # -3e38 where masked (added into the QK PSUM via an identity matmul)
    p = np.arange(128)[:, None]
    f = np.arange(512)[None, :]
    mask = np.stack([np.where(128 * r + p <= f, 0.0, -3e38).astype(F32)
                     for r in range(4)],
                    axis=1).reshape(128, 4 * 512).astype(BF16)

    ident = np.eye(128, dtype=np.float32).astype(BF16)
    onem = np.ones((128, 128), BF16)

    in_maps = []
    for g in range(N_CORES):
        # block-major: blk in (q0..q3, k, v), each [mc, 128p, 128cols]
        cols = [Wqkv[:, 512 * g + 128 * j:512 * g + 128 * (j + 1)]
                for j in range(G)]
        cols.append(Wqkv[:, D + 128 * g:D + 128 * (g + 1)])
        cols.append(Wqkv[:, D + KV + 128 * g:D + KV + 128 * (g + 1)])
        wqkv_g = np.ascontiguousarray(
            np.stack([c.reshape(MC, 128, 128) for c in cols], axis=0)
            .transpose(2, 0, 1, 3)   # [128p, blk, mc, 128]
        ).reshape(128, 6 * MC * 128).astype(BF16)
        wo_g = np.ascontiguousarray(
            Wo[512 * g:512 * (g + 1), :].reshape(G, 128, 8, 512)
            .transpose(1, 0, 2, 3)).reshape(128, G * 8 * 512).astype(BF16)
        bqw_g = np.concatenate([
            bqkv[512 * g:512 * (g + 1)],
            bqkv[D + 128 * g:D + 128 * (g + 1)],
            bqkv[D + KV + 128 * g:D + KV + 128 * (g + 1)],
        ]).reshape(1, 6 * 128).astype(BF16)
        in_maps.append({
            "xt": xt_t, "wqkv": wqkv_g, "wo": wo_g,
            "cos": cos_h, "sin": sin_h,
            "bqw": bqw_g, "ones": np.ones((1, 512), BF16), "mask": mask,
            "ident": ident, "onem": onem,
        })
    return in_maps


def kernel(x, Wqkv, bqkv, Wo, bo):
    hasbias = bool(np.any(np.asarray(bqkv)))
    key = ("nc", hasbias)
    if key not in _CACHE:
        _CACHE[key] = _build(reps=1, hasbias=hasbias)
    nc = _CACHE[key]
    in_maps = _prep_inputs(x, Wqkv, bqkv, Wo, bo)
    res = run_bass_kernel_spmd(nc, in_maps, core_ids=list(range(N_CORES)))
    # reassemble: core g's out block b (RSOUT rows) = final rows
    # [RSROWS*b + RSOUT*g : +RSOUT]
    out = np.zeros((S, D), F32)
    for g in range(N_CORES):
        og = np.asarray(res.results[g]["out"], F32)   # [SSLICE, D]
        for b in range(RSC):
            out[RSROWS * b + RSOUT * g:
                RSROWS * b + RSOUT * (g + 1)] = og[b * RSOUT:(b + 1) * RSOUT]
    out = out + np.asarray(bo, F32)[None, :]
    return out[None].astype(F32)


# revision 3
# speedup vs baseline: 1.0121x; 1.0121x over previous
"""GroupedQueryAttention (B=1, S=2048, D=4096, 32 Q / 8 KV heads) on 8 TRN2 cores.

Fused single-scope pipeline: QKV projection, RoPE, causal attention and
out-projection partials share one pool set (8 PSUM banks total), so the
Tile scheduler can fill attention dependency stalls with projection /
out-proj matmuls.  ReduceScatter is row-chunked: chunk b ([S/RSC, D]) is
issued as soon as its out-proj rows are staged, overlapping the collective
with remaining compute.  Host reassembles the row-interleaved RS outputs.

Per core (one KV group = 4 Q heads + 1 KV head):
  - QKV projection in 3 waves of 2 PSUM chains (q01, q23, kv), mc-outer
    so each weight/x chunk is consumed right after its DMA lands
  - RoPE on q and k via DVE (scale 1/sqrt(HD) split sqrt-wise onto both)
  - causal flash attention in scores^T orientation, diagonal tiles trimmed
  - out-proj partials against own 512 Wo rows, staged as [128,4096] rows
    to DRAM (1MB DMAs on the ACT HWDGE ring), row-chunked ReduceScatter
"""
import numpy as np
import ml_dtypes

from concourse import bass, bacc, tile, mybir
from concourse.bass_utils import run_bass_kernel_spmd

BF16 = ml_dtypes.bfloat16
F32 = np.float32

D = 4096          # model dim
S = 2048          # sequence
NH = 32           # query heads
NG = 8            # kv heads == n cores
HD = 128          # head dim
G = NH // NG      # 4 query heads per group/core
KV = NG * HD      # 1024
BASE = 50000.0
SCALE = 1.0 / np.sqrt(HD)
N_CORES = 8
SC = S // 512     # 4 s-chunks of 512
MC = D // 128     # 32 contraction chunks
HALF = MC // 2    # x chunk split for SBUF
SSLICE = S // N_CORES  # 256 rows of final output per core
RSC = 8           # reduce-scatter row chunks
RSROWS = S // RSC         # 512 rows per RS chunk
RSOUT = RSROWS // N_CORES  # 64 rows per core per RS chunk
STS_PER_RS = 16 // RSC     # st row-blocks per RS chunk

_CACHE = {}


def _build(reps: int = 1, sim: bool = False, coll: str = "rs",
           nodma: bool = False, trim: bool = True, rsc: int = RSC,
           hasbias: bool = True, noattn: bool = False, ilv: bool = True):
    if sim:
        coll = "copy"
    sts_per = 16 // rsc
    rsrows = S // rsc
    rsout = rsrows // N_CORES

    def dma(dst, src, **kwargs):
        if not nodma:
            nc.sync.dma_start(dst, src, **kwargs)
        else:
            # timing probe: touch the tile with a tiny transfer so Tile
            # sees it written; data is garbage but schedule shape holds
            nc.sync.dma_start(dst[:, 0:64], src[:, 0:64], **kwargs)

    def dma_store(*args, **kwargs):
        # stores ride the ACT HWDGE ring so they never queue ahead of loads
        nc.scalar.dma_start(*args, **kwargs)
    f32 = mybir.dt.float32
    bf16 = mybir.dt.bfloat16

    nc = bacc.Bacc("TRN2", target_bir_lowering=False, debug=False,
                   num_devices=N_CORES)

    # ---- I/O ----
    xt_d = nc.dram_tensor("xt", [128, SC * MC * 512], bf16, kind="ExternalInput")
    # block-major qkv weights: [128, (blk, mc, 128)], blk = q0..q3, k, v
    wqkv_d = nc.dram_tensor("wqkv", [128, 6 * MC * 128], bf16,
                            kind="ExternalInput")
    wo_d = nc.dram_tensor("wo", [128, G * 8 * 512], bf16, kind="ExternalInput")
    cos_d = nc.dram_tensor("cos", [128, S], bf16, kind="ExternalInput")
    sin_d = nc.dram_tensor("sin", [128, S], bf16, kind="ExternalInput")
    bqw_d = nc.dram_tensor("bqw", [1, 6 * 128], bf16, kind="ExternalInput")
    ones_d = nc.dram_tensor("ones", [1, 512], bf16, kind="ExternalInput")
    mask_d = nc.dram_tensor("mask", [128, 4 * 512], bf16, kind="ExternalInput")
    ident_d = nc.dram_tensor("ident", [128, 128], bf16, kind="ExternalInput")
    onem_d = nc.dram_tensor("onem", [128, 128], bf16, kind="ExternalInput")
    out_d = nc.dram_tensor("out", [SSLICE, D], bf16, kind="ExternalOutput")

    Ident = mybir.ActivationFunctionType.Identity
    CopyF = mybir.ActivationFunctionType.Copy
    Exp = mybir.ActivationFunctionType.Exp
    mult = mybir.AluOpType.mult

    from contextlib import ExitStack
    with tile.TileContext(nc) as tc:
        with ExitStack() as stack:
            ent = stack.enter_context
            cp = ent(tc.tile_pool(name="const", bufs=1))
            wp = ent(tc.tile_pool(name="wts", bufs=1))
            pp = ent(tc.tile_pool(name="pers", bufs=1))
            qtp = ent(tc.tile_pool(name="qt", bufs=2))
            atp = ent(tc.tile_pool(name="at", bufs=2))
            xsp = ent(tc.tile_pool(name="xs", bufs=6))
            vtp = ent(tc.tile_pool(name="vt", bufs=2))
            rtp = ent(tc.tile_pool(name="rt", bufs=2))
            esp = ent(tc.tile_pool(name="es", bufs=4))
            recp = ent(tc.tile_pool(name="rc", bufs=2))
            osp = ent(tc.tile_pool(name="os", bufs=2))
            qkvp = ent(tc.tile_pool(name="qkv_ps", bufs=2, space="PSUM"))
            qkp = ent(tc.tile_pool(name="qk_ps", bufs=1, space="PSUM"))
            avp = ent(tc.tile_pool(name="av_ps", bufs=1, space="PSUM"))
            bpp = ent(tc.tile_pool(name="bp_ps", bufs=1, space="PSUM"))
            pop = ent(tc.tile_pool(name="po_ps", bufs=2, space="PSUM"))
            dramp = ent(tc.tile_pool(name="dram", bufs=1, space="DRAM"))
            # constants: loaded once
            bqw = cp.tile([1, 6 * 128], bf16)
            nc.sync.dma_start(bqw[:], bqw_d[:])
            onesr = cp.tile([1, 512], bf16); nc.sync.dma_start(onesr[:], ones_d[:])
            mask = cp.tile([128, 4 * 512], bf16); nc.sync.dma_start(mask[:], mask_d[:])
            ident = cp.tile([128, 128], bf16); nc.sync.dma_start(ident[:], ident_d[:])
            onem = cp.tile([128, 128], bf16); nc.sync.dma_start(onem[:], onem_d[:])
            cos = cp.tile([128, S], bf16); nc.sync.dma_start(cos[:], cos_d[:])
            sin = cp.tile([128, S], bf16); nc.sync.dma_start(sin[:], sin_d[:])

            for _rep in range(reps):
                # per-rep weight tiles (ring reuse serializes reps naturally)
                wqkv = wp.tile([128, 6 * MC * 128], bf16, name="wqkv")
                wo = wp.tile([128, G * 8 * 512], bf16, name="wo")
                kT = pp.tile([128, S], bf16, name="kT")
                vN = pp.tile([128, S], bf16, name="vN")

                # rep-start loads in consumption order (SP HWDGE ring FIFO)
                xh = {}
                QTR = MC // 4

                def load_xq(sc, i):
                    xh[(sc, i)] = xsp.tile([128, QTR * 512], bf16, name="xs")
                    dma(xh[(sc, i)][:],
                        xt_d[:, (sc * MC + i * QTR) * 512:
                             (sc * MC + (i + 1) * QTR) * 512])

                def load_x(sc):
                    for i in range(4):
                        load_xq(sc, i)

                # interleave x quarters with weight blocks in consumption
                # order (k, v chains run first)
                for i, j in ((0, 4), (1, 5), (2, 0), (3, 1)):
                    load_xq(0, i)
                    dma(wqkv[:, j * 4096:(j + 1) * 4096],
                        wqkv_d[:, j * 4096:(j + 1) * 4096])
                for j in (2, 3):
                    dma(wqkv[:, j * 4096:(j + 1) * 4096],
                        wqkv_d[:, j * 4096:(j + 1) * 4096])

                attnT = {}   # chunk -> tile handle

                def rope(dst, ps, sc):
                    # dst = ps*cos + swap64(ps)*sin_signed  (tables pre-scaled)
                    cs = slice(sc * 512, (sc + 1) * 512)
                    t1 = rtp.tile([128, 512], bf16, name="rt")
                    nc.vector.tensor_tensor(t1[:], ps[:], cos[:, cs], mult)
                    t2 = rtp.tile([128, 512], bf16, name="rt")
                    nc.vector.tensor_tensor(t2[0:64, :], ps[64:128, :],
                                            sin[0:64, cs], mult)
                    nc.vector.tensor_tensor(t2[64:128, :], ps[0:64, :],
                                            sin[64:128, cs], mult)
                    nc.vector.tensor_add(dst, t1[:], t2[:])

                def qkv_chunk(sc):
                    # 6 sequential chains (q0..q3, k, v): rope of chain j
                    # drains while chains j+1, j+2 run -> no PSUM WAR stall
                    qTc = qtp.tile([128, G * 512], bf16, name="qT")
                    cs = slice(sc * 512, (sc + 1) * 512)
                    for blk in (4, 5, 0, 1, 2, 3):  # k,v first: attention
                        # chunk c needs kT/vN before its per-head qT ropes
                        ps = qkvp.tile([128, 512], f32, name="qkv")
                        if hasbias:
                            nc.tensor.matmul(ps[:],
                                             bqw[:, blk * 128:(blk + 1) * 128],
                                             onesr[:], start=True, stop=False)
                        for mc in range(MC):
                            xsl = xh[(sc, mc // QTR)][
                                :, (mc % QTR) * 512:(mc % QTR) * 512 + 512]
                            lhs = wqkv[:, (blk * MC + mc) * 128:
                                       (blk * MC + mc) * 128 + 128]
                            nc.tensor.matmul(ps[:], lhs, xsl,
                                             start=(mc == 0 and not hasbias),
                                             stop=(mc == MC - 1))
                        if blk < 4:
                            rope(qTc[:, blk * 512:blk * 512 + 512], ps, sc)
                        elif blk == 4:
                            rope(kT[:, cs], ps, sc)
                        else:
                            vTt = vtp.tile([128, 512], bf16, name="vt")
                            nc.vector.tensor_copy(vTt[:], ps[:])
                            pst = pop.tile([128, 512], bf16, name="po")
                            for i in range(4):
                                nc.tensor.transpose(
                                    pst[:, i * 128:(i + 1) * 128],
                                    vTt[:, i * 128:(i + 1) * 128], ident[:])
                            nc.vector.tensor_copy(vN[:, cs], pst[:])
                    return qTc

                def piece_ep(pc, st, ep, osb):
                    # one out-proj ep unit: 8 matmuls + 2 copies; stages the
                    # finished [128, D] row-block after its last ep
                    r = st - 4 * pc
                    aT = attnT[pc]
                    e0, e1 = 2 * ep, 2 * ep + 1
                    poA = pop.tile([128, 512], f32, name="po")
                    poB = pop.tile([128, 512], f32, name="po")
                    for h in range(G):
                        lw = aT[:, h * 512 + r * 128:h * 512 + r * 128 + 128]
                        nc.tensor.matmul(
                            poA[:], lw,
                            wo[:, (h * 8 + e0) * 512:(h * 8 + e0) * 512 + 512],
                            start=(h == 0), stop=(h == G - 1))
                        nc.tensor.matmul(
                            poB[:], lw,
                            wo[:, (h * 8 + e1) * 512:(h * 8 + e1) * 512 + 512],
                            start=(h == 0), stop=(h == G - 1))
                    # keep ACT exp-only: both copies on DVE
                    nc.vector.tensor_copy(osb[:, e0 * 512:(e0 + 1) * 512],
                                          poA[:])
                    nc.vector.tensor_copy(osb[:, e1 * 512:(e1 + 1) * 512],
                                          poB[:])
                    if ep == 3:
                        if coll != "noout" and not nodma:
                            b = st // sts_per
                            rr = st % sts_per
                            dma_store(rs_in[b][rr * 128:(rr + 1) * 128, :],
                                      osb[:])
                        if (st + 1) % sts_per == 0:
                            issue_rs(st // sts_per)

                pending = []

                def queue_piece(pc, st):
                    osb = osp.tile([128, D], bf16, name="osb")
                    for ep in range(4):
                        pending.append((piece_ep, pc, st, ep, osb))

                def drain_piece(n=None):
                    k = len(pending) if n is None else min(n, len(pending))
                    for _ in range(k):
                        f, *a = pending.pop(0)
                        f(*a)

                def issue_rs(b):
                    if coll == "noout":
                        return
                    if coll == "copy":
                        dma_store(out_d[b * rsout:(b + 1) * rsout, :],
                                  rs_in[b][0:rsout, :])
                        return
                    ro = dramp.tile([rsout, D], bf16, name=f"rs_out{b}", bufs=2)
                    nc.gpsimd.collective_compute(
                        "ReduceScatter", mybir.AluOpType.add,
                        replica_groups=[list(range(N_CORES))],
                        ins=[rs_in[b].opt()], outs=[ro.opt()])
                    dma_store(out_d[b * rsout:(b + 1) * rsout, :], ro[:])

                def attn_head(sc, h, qTc):
                    # chain per t-tile: QK (+mask-add via identity matmul on
                    # diagonal tiles) -> exp (ACT) -> AV + denominator, both
                    # accumulated on PE.  No DVE op anywhere in the chain.
                    hs = h * 512
                    ntt = 4 * (sc + 1)
                    av = avp.tile([128, 512], f32, name="av")
                    bps = bpp.tile([128, 512], f32, name="bp")

                    def emit_qk(tb):
                        qk = qkp.tile([128, 1024], f32, name="qk")
                        q0s = []
                        for q2 in range(2):
                            tt = 2 * tb + q2
                            r = tt - 4 * sc
                            q0 = max(r, 0) * 128 if trim else 0
                            q0s.append(q0)
                            sl = slice(q2 * 512 + q0, (q2 + 1) * 512)
                            nc.tensor.matmul(
                                qk[:, sl],
                                kT[:, tt * 128:(tt + 1) * 128],
                                qTc[:, hs + q0:hs + 512],
                                start=True, stop=(r < 0))
                            if r >= 0:
                                # fold causal mask in: qk += I.T @ (-3e38
                                # staircase), only the 128-wide diagonal window
                                nc.tensor.matmul(
                                    qk[:, q2 * 512 + q0:q2 * 512 + q0 + 128],
                                    ident[:],
                                    mask[:, r * 512 + q0:r * 512 + q0 + 128],
                                    start=False, stop=True)
                        es = esp.tile([128, 1024], bf16, name="es")
                        if q0s[1] > 0:
                            for q2 in range(2):
                                q0 = q0s[q2]
                                nc.scalar.activation(
                                    es[:, q2 * 512 + q0:(q2 + 1) * 512],
                                    qk[:, q2 * 512 + q0:(q2 + 1) * 512], Exp)
                        else:
                            nc.scalar.activation(es[:], qk[:], Exp)
                        return es, q0s

                    def emit_av(tb, es, q0s):
                        for q2 in range(2):
                            tt = 2 * tb + q2
                            q0 = q0s[q2]
                            esl = es[:, q2 * 512 + q0:(q2 + 1) * 512]
                            nc.tensor.matmul(
                                av[:, q0:512], vN[:, tt * 128:(tt + 1) * 128],
                                esl, start=(tt == 0), stop=(tt == ntt - 1))
                            nc.tensor.matmul(
                                bps[:, q0:512], onem[:], esl,
                                start=(tt == 0), stop=(tt == ntt - 1))

                    prev = None
                    for tb in range(ntt // 2):
                        cur = (tb, *emit_qk(tb))
                        if prev is not None:
                            emit_av(*prev)
                        if ilv:
                            drain_piece(1)
                        prev = cur
                    emit_av(*prev)
                    recb = recp.tile([128, 512], f32, name="rec")
                    nc.vector.reciprocal(recb[:], bps[:])
                    nc.vector.tensor_tensor(attnT[sc][:, hs:hs + 512], av[:],
                                            recb[:], mult)

                if coll != "noout":
                    rs_in = [dramp.tile([rsrows, D], bf16, name=f"rs_in{b}",
                                        bufs=2) for b in range(rsc)]

                for sc in range(SC):
                    qTc = qkv_chunk(sc)
                    attnT[sc] = atp.tile([128, G * 512], bf16, name="attnT")
                    # prefetch next x chunk; wo after first chunk's x
                    if sc < SC - 1:
                        load_x(sc + 1)
                    if sc == 0 and not noattn:
                        for j in range(4):
                            dma(wo[:, j * 4096:(j + 1) * 4096],
                                wo_d[:, j * 4096:(j + 1) * 4096])
                    if noattn:
                        continue
                    for h in range(G):
                        attn_head(sc, h, qTc)
                        if sc >= 1:
                            queue_piece(sc - 1, 4 * (sc - 1) + h)
                            if not ilv:
                                drain_piece()
                    drain_piece()
                if noattn:
                    # consume qT/attnT rings so Tile sees them written+read
                    for sc in range(SC):
                        nc.vector.tensor_copy(attnT[sc][:, 0:512],
                                              kT[:, 0:512])
                    continue
                for st in range(12, 16):
                    queue_piece(3, st)
                drain_piece()
    nc.compile()
    return nc


def _prep_inputs(x, Wqkv, bqkv, Wo, bo):
    """Host-side shard prep. Returns in_maps for the 8 cores."""
    x0T = np.ascontiguousarray(np.asarray(x, F32)[0].T)          # [D, S]
    xt_t = np.ascontiguousarray(
        x0T.reshape(MC, 128, SC, 512).transpose(1, 2, 0, 3)
    ).reshape(128, SC * MC * 512).astype(BF16)

    Wqkv = np.asarray(Wqkv, F32)
    Wo = np.asarray(Wo, F32)
    bqkv = np.asarray(bqkv, F32)

    # rope tables (transposed [HD, S]); 1/sqrt(HD) split sqrt-wise onto q & k
    inv_freq = 1.0 / (BASE ** (np.arange(0, HD, 2, dtype=np.float64) / HD))
    t = np.arange(S, dtype=np.float64)
    freqs = np.outer(t, inv_freq)                                # [S, 64]
    emb = np.concatenate([freqs, freqs], axis=1)                 # [S, HD]
    s4 = SCALE ** 0.5
    cosT = (np.cos(emb).T * s4).astype(F32)                      # [HD, S]
    sinT = (np.sin(emb).T * s4).astype(F32)
    sin_signed = np.concatenate([-sinT[:64], sinT[64:]], axis=0)
    cos_h = np.ascontiguousarray(cosT).astype(BF16)
    sin_h = np.ascontiguousarray(sin_signed).astype(BF16)

    # additive causal masks for the 4 diagonal t-tiles: 0 where visible,
    # -3e38 where masked (added into the QK PSUM via an identity matmul)
    p = np.arange(128)[:, None]
    f = np.arange(512)[None, :]
    mask = np.stack([np.where(128 * r + p <= f, 0.0, -3e38).astype(F32)
                     for r in range(4)],
                    axis=1).reshape(128, 4 * 512).astype(BF16)

    ident = np.eye(128, dtype=np.float32).astype(BF16)
    onem = np.ones((128, 128), BF16)

    in_maps = []
    for g in range(N_CORES):
        # block-major: blk in (q0..q3, k, v), each [mc, 128p, 128cols]
        cols = [Wqkv[:, 512 * g + 128 * j:512 * g + 128 * (j + 1)]
                for j in range(G)]
        cols.append(Wqkv[:, D + 128 * g:D + 128 * (g + 1)])
        cols.append(Wqkv[:, D + KV + 128 * g:D + KV + 128 * (g + 1)])
        wqkv_g = np.ascontiguousarray(
            np.stack([c.reshape(MC, 128, 128) for c in cols], axis=0)
            .transpose(2, 0, 1, 3)   # [128p, blk, mc, 128]
        ).reshape(128, 6 * MC * 128).astype(BF16)
        wo_g = np.ascontiguousarray(
            Wo[512 * g:512 * (g + 1), :].reshape(G, 128, 8, 512)
            .transpose(1, 0, 2, 3)).reshape(128, G * 8 * 512).astype(BF16)
        bqw_g = np.concatenate([
            bqkv[512 * g:512 * (g + 1)],
            bqkv[D + 128 * g:D + 128 * (g + 1)],
            bqkv[D + KV + 128 * g:D + KV + 128 * (g + 1)],
        ]).reshape(1, 6 * 128).astype(BF16)
        in_maps.append({
            "xt": xt_t, "wqkv": wqkv_g, "wo": wo_g,
            "cos": cos_h, "sin": sin_h,
            "bqw": bqw_g, "ones": np.ones((1, 512), BF16), "mask": mask,
            "ident": ident, "onem": onem,
        })
    return in_maps


def kernel(x, Wqkv, bqkv, Wo, bo):
    hasbias = bool(np.any(np.asarray(bqkv)))
    key = ("nc", hasbias)
    if key not in _CACHE:
        _CACHE[key] = _build(reps=1, hasbias=hasbias)
    nc = _CACHE[key]
    in_maps = _prep_inputs(x, Wqkv, bqkv, Wo, bo)
    res = run_bass_kernel_spmd(nc, in_maps, core_ids=list(range(N_CORES)))
    # reassemble: core g's out block b (RSOUT rows) = final rows
    # [RSROWS*b + RSOUT*g : +RSOUT]
    out = np.zeros((S, D), F32)
    for g in range(N_CORES):
        og = np.asarray(res.results[g]["out"], F32)   # [SSLICE, D]
        for b in range(RSC):
            out[RSROWS * b + RSOUT * g:
                RSROWS * b + RSOUT * (g + 1)] = og[b * RSOUT:(b + 1) * RSOUT]
    out = out + np.asarray(bo, F32)[None, :]
    return out[None].astype(F32)


# revision 4
# speedup vs baseline: 1.0469x; 1.0344x over previous
"""GroupedQueryAttention (B=1, S=2048, D=4096, 32 Q / 8 KV heads) on 8 TRN2 cores.

Fused single-scope pipeline: QKV projection, RoPE, causal attention and
out-projection partials share one pool set (8 PSUM banks total), so the
Tile scheduler can fill attention dependency stalls with projection /
out-proj matmuls.  ReduceScatter is row-chunked: chunk b ([S/RSC, D]) is
issued as soon as its out-proj rows are staged, overlapping the collective
with remaining compute.  Host reassembles the row-interleaved RS outputs.

Per core (one KV group = 4 Q heads + 1 KV head):
  - QKV projection in 3 waves of 2 PSUM chains (q01, q23, kv), mc-outer
    so each weight/x chunk is consumed right after its DMA lands
  - RoPE on q and k via DVE (scale 1/sqrt(HD) split sqrt-wise onto both)
  - causal flash attention in scores^T orientation, diagonal tiles trimmed
  - out-proj partials against own 512 Wo rows, staged as [128,4096] rows
    to DRAM (1MB DMAs on the ACT HWDGE ring), row-chunked ReduceScatter
"""
import numpy as np
import ml_dtypes

from concourse import bass, bacc, tile, mybir
from concourse.bass_utils import run_bass_kernel_spmd

BF16 = ml_dtypes.bfloat16
F32 = np.float32

D = 4096          # model dim
S = 2048          # sequence
NH = 32           # query heads
NG = 8            # kv heads == n cores
HD = 128          # head dim
G = NH // NG      # 4 query heads per group/core
KV = NG * HD      # 1024
BASE = 50000.0
SCALE = 1.0 / np.sqrt(HD)
N_CORES = 8
SC = S // 512     # 4 s-chunks of 512
MC = D // 128     # 32 contraction chunks
HALF = MC // 2    # x chunk split for SBUF
SSLICE = S // N_CORES  # 256 rows of final output per core
RSC = 16          # reduce-scatter row chunks
RSROWS = S // RSC         # 512 rows per RS chunk
RSOUT = RSROWS // N_CORES  # 64 rows per core per RS chunk
STS_PER_RS = 16 // RSC     # st row-blocks per RS chunk

_CACHE = {}


def _build(reps: int = 1, sim: bool = False, coll: str = "rs",
           nodma: bool = False, trim: bool = True, rsc: int = RSC,
           hasbias: bool = True, noattn: bool = False, ilv: bool = True):
    if sim:
        coll = "copy"
    sts_per = 16 // rsc
    rsrows = S // rsc
    rsout = rsrows // N_CORES

    def dma(dst, src, **kwargs):
        if not nodma:
            nc.sync.dma_start(dst, src, **kwargs)
        else:
            # timing probe: touch the tile with a tiny transfer so Tile
            # sees it written; data is garbage but schedule shape holds
            nc.sync.dma_start(dst[:, 0:64], src[:, 0:64], **kwargs)

    def dma_store(*args, **kwargs):
        # stores ride the ACT HWDGE ring so they never queue ahead of loads
        nc.scalar.dma_start(*args, **kwargs)
    f32 = mybir.dt.float32
    bf16 = mybir.dt.bfloat16

    nc = bacc.Bacc("TRN2", target_bir_lowering=False, debug=False,
                   num_devices=N_CORES)

    # ---- I/O ----
    xt_d = nc.dram_tensor("xt", [128, SC * MC * 512], bf16, kind="ExternalInput")
    # block-major qkv weights: [128, (blk, mc, 128)], blk = q0..q3, k, v
    wqkv_d = nc.dram_tensor("wqkv", [128, 6 * MC * 128], bf16,
                            kind="ExternalInput")
    wo_d = nc.dram_tensor("wo", [128, G * 8 * 512], bf16, kind="ExternalInput")
    cos_d = nc.dram_tensor("cos", [128, S], bf16, kind="ExternalInput")
    sin_d = nc.dram_tensor("sin", [128, S], bf16, kind="ExternalInput")
    bqw_d = nc.dram_tensor("bqw", [1, 6 * 128], bf16, kind="ExternalInput")
    ones_d = nc.dram_tensor("ones", [1, 512], bf16, kind="ExternalInput")
    mask_d = nc.dram_tensor("mask", [128, 4 * 512], bf16, kind="ExternalInput")
    ident_d = nc.dram_tensor("ident", [128, 128], bf16, kind="ExternalInput")
    onem_d = nc.dram_tensor("onem", [128, 128], bf16, kind="ExternalInput")
    out_d = nc.dram_tensor("out", [SSLICE, D], bf16, kind="ExternalOutput")

    Ident = mybir.ActivationFunctionType.Identity
    CopyF = mybir.ActivationFunctionType.Copy
    Exp = mybir.ActivationFunctionType.Exp
    mult = mybir.AluOpType.mult

    from contextlib import ExitStack
    with tile.TileContext(nc) as tc:
        with ExitStack() as stack:
            ent = stack.enter_context
            cp = ent(tc.tile_pool(name="const", bufs=1))
            wp = ent(tc.tile_pool(name="wts", bufs=1))
            pp = ent(tc.tile_pool(name="pers", bufs=1))
            qtp = ent(tc.tile_pool(name="qt", bufs=2))
            atp = ent(tc.tile_pool(name="at", bufs=2))
            xsp = ent(tc.tile_pool(name="xs", bufs=6))
            vtp = ent(tc.tile_pool(name="vt", bufs=2))
            rtp = ent(tc.tile_pool(name="rt", bufs=2))
            esp = ent(tc.tile_pool(name="es", bufs=6))
            recp = ent(tc.tile_pool(name="rc", bufs=2))
            osp = ent(tc.tile_pool(name="os", bufs=2))
            qkvp = ent(tc.tile_pool(name="qkv_ps", bufs=2, space="PSUM"))
            qkp = ent(tc.tile_pool(name="qk_ps", bufs=1, space="PSUM"))
            avp = ent(tc.tile_pool(name="av_ps", bufs=1, space="PSUM"))
            bpp = ent(tc.tile_pool(name="bp_ps", bufs=1, space="PSUM"))
            pop = ent(tc.tile_pool(name="po_ps", bufs=2, space="PSUM"))
            dramp = ent(tc.tile_pool(name="dram", bufs=1, space="DRAM"))
            # constants: loaded once
            bqw = cp.tile([1, 6 * 128], bf16)
            nc.sync.dma_start(bqw[:], bqw_d[:])
            onesr = cp.tile([1, 512], bf16); nc.sync.dma_start(onesr[:], ones_d[:])
            mask = cp.tile([128, 4 * 512], bf16); nc.sync.dma_start(mask[:], mask_d[:])
            ident = cp.tile([128, 128], bf16); nc.sync.dma_start(ident[:], ident_d[:])
            onem = cp.tile([128, 128], bf16); nc.sync.dma_start(onem[:], onem_d[:])
            cos = cp.tile([128, S], bf16); nc.sync.dma_start(cos[:], cos_d[:])
            sin = cp.tile([128, S], bf16); nc.sync.dma_start(sin[:], sin_d[:])

            for _rep in range(reps):
                # per-rep weight tiles (ring reuse serializes reps naturally)
                wqkv = wp.tile([128, 6 * MC * 128], bf16, name="wqkv")
                wo = wp.tile([128, G * 8 * 512], bf16, name="wo")
                kT = pp.tile([128, S], bf16, name="kT")
                vN = pp.tile([128, S], bf16, name="vN")

                # rep-start loads in consumption order (SP HWDGE ring FIFO)
                xh = {}
                QTR = MC // 4

                def load_xq(sc, i):
                    xh[(sc, i)] = xsp.tile([128, QTR * 512], bf16, name="xs")
                    dma(xh[(sc, i)][:],
                        xt_d[:, (sc * MC + i * QTR) * 512:
                             (sc * MC + (i + 1) * QTR) * 512])

                def load_x(sc):
                    for i in range(4):
                        load_xq(sc, i)

                # interleave x quarters with weight blocks in consumption
                # order (k, v chains run first)
                for i, j in ((0, 4), (1, 5), (2, 0), (3, 1)):
                    load_xq(0, i)
                    dma(wqkv[:, j * 4096:(j + 1) * 4096],
                        wqkv_d[:, j * 4096:(j + 1) * 4096])
                for j in (2, 3):
                    dma(wqkv[:, j * 4096:(j + 1) * 4096],
                        wqkv_d[:, j * 4096:(j + 1) * 4096])

                attnT = {}   # chunk -> tile handle

                def rope(dst, ps, sc):
                    # dst = ps*cos + swap64(ps)*sin_signed  (tables pre-scaled)
                    cs = slice(sc * 512, (sc + 1) * 512)
                    t1 = rtp.tile([128, 512], bf16, name="rt")
                    nc.vector.tensor_tensor(t1[:], ps[:], cos[:, cs], mult)
                    t2 = rtp.tile([128, 512], bf16, name="rt")
                    nc.vector.tensor_tensor(t2[0:64, :], ps[64:128, :],
                                            sin[0:64, cs], mult)
                    nc.vector.tensor_tensor(t2[64:128, :], ps[0:64, :],
                                            sin[64:128, cs], mult)
                    nc.vector.tensor_add(dst, t1[:], t2[:])

                def qkv_chunk(sc):
                    # 6 sequential chains (q0..q3, k, v): rope of chain j
                    # drains while chains j+1, j+2 run -> no PSUM WAR stall
                    qTc = qtp.tile([128, G * 512], bf16, name="qT")
                    cs = slice(sc * 512, (sc + 1) * 512)
                    for blk in (4, 5, 0, 1, 2, 3):  # k,v first: attention
                        # chunk c needs kT/vN before its per-head qT ropes
                        ps = qkvp.tile([128, 512], f32, name="qkv")
                        if hasbias:
                            nc.tensor.matmul(ps[:],
                                             bqw[:, blk * 128:(blk + 1) * 128],
                                             onesr[:], start=True, stop=False)
                        for mc in range(MC):
                            xsl = xh[(sc, mc // QTR)][
                                :, (mc % QTR) * 512:(mc % QTR) * 512 + 512]
                            lhs = wqkv[:, (blk * MC + mc) * 128:
                                       (blk * MC + mc) * 128 + 128]
                            nc.tensor.matmul(ps[:], lhs, xsl,
                                             start=(mc == 0 and not hasbias),
                                             stop=(mc == MC - 1))
                        if blk < 4:
                            rope(qTc[:, blk * 512:blk * 512 + 512], ps, sc)
                        elif blk == 4:
                            rope(kT[:, cs], ps, sc)
                        else:
                            vTt = vtp.tile([128, 512], bf16, name="vt")
                            nc.vector.tensor_copy(vTt[:], ps[:])
                            pst = pop.tile([128, 512], bf16, name="po")
                            for i in range(4):
                                nc.tensor.transpose(
                                    pst[:, i * 128:(i + 1) * 128],
                                    vTt[:, i * 128:(i + 1) * 128], ident[:])
                            nc.vector.tensor_copy(vN[:, cs], pst[:])
                    return qTc

                def piece_ep(pc, st, ep, osb):
                    # one out-proj ep unit: 8 matmuls + 2 copies; stages the
                    # finished [128, D] row-block after its last ep
                    r = st - 4 * pc
                    aT = attnT[pc]
                    e0, e1 = 2 * ep, 2 * ep + 1
                    poA = pop.tile([128, 512], f32, name="po")
                    poB = pop.tile([128, 512], f32, name="po")
                    for h in range(G):
                        lw = aT[:, h * 512 + r * 128:h * 512 + r * 128 + 128]
                        nc.tensor.matmul(
                            poA[:], lw,
                            wo[:, (h * 8 + e0) * 512:(h * 8 + e0) * 512 + 512],
                            start=(h == 0), stop=(h == G - 1))
                        nc.tensor.matmul(
                            poB[:], lw,
                            wo[:, (h * 8 + e1) * 512:(h * 8 + e1) * 512 + 512],
                            start=(h == 0), stop=(h == G - 1))
                    # keep ACT exp-only: both copies on DVE
                    nc.vector.tensor_copy(osb[:, e0 * 512:(e0 + 1) * 512],
                                          poA[:])
                    nc.vector.tensor_copy(osb[:, e1 * 512:(e1 + 1) * 512],
                                          poB[:])
                    if ep == 3:
                        if coll != "noout" and not nodma:
                            b = st // sts_per
                            rr = st % sts_per
                            dma_store(rs_in[b][rr * 128:(rr + 1) * 128, :],
                                      osb[:])
                        if (st + 1) % sts_per == 0:
                            issue_rs(st // sts_per)

                pending = []

                def queue_piece(pc, st):
                    osb = osp.tile([128, D], bf16, name="osb")
                    for ep in range(4):
                        pending.append((piece_ep, pc, st, ep, osb))

                def drain_piece(n=None):
                    k = len(pending) if n is None else min(n, len(pending))
                    for _ in range(k):
                        f, *a = pending.pop(0)
                        f(*a)

                def issue_rs(b):
                    if coll == "noout":
                        return
                    if coll == "copy":
                        dma_store(out_d[b * rsout:(b + 1) * rsout, :],
                                  rs_in[b][0:rsout, :])
                        return
                    ro = dramp.tile([rsout, D], bf16, name=f"rs_out{b}", bufs=2)
                    nc.gpsimd.collective_compute(
                        "ReduceScatter", mybir.AluOpType.add,
                        replica_groups=[list(range(N_CORES))],
                        ins=[rs_in[b].opt()], outs=[ro.opt()])
                    dma_store(out_d[b * rsout:(b + 1) * rsout, :], ro[:])

                def attn_head(sc, h, qTc):
                    # chain per t-tile: QK (+mask-add via identity matmul on
                    # diagonal tiles) -> exp (ACT) -> AV + denominator, both
                    # accumulated on PE.  No DVE op anywhere in the chain.
                    hs = h * 512
                    ntt = 4 * (sc + 1)
                    av = avp.tile([128, 512], f32, name="av")
                    bps = bpp.tile([128, 512], f32, name="bp")

                    def emit_qk(tb):
                        qk = qkp.tile([128, 1024], f32, name="qk")
                        q0s = []
                        for q2 in range(2):
                            tt = 2 * tb + q2
                            r = tt - 4 * sc
                            q0 = max(r, 0) * 128 if trim else 0
                            q0s.append(q0)
                            sl = slice(q2 * 512 + q0, (q2 + 1) * 512)
                            nc.tensor.matmul(
                                qk[:, sl],
                                kT[:, tt * 128:(tt + 1) * 128],
                                qTc[:, hs + q0:hs + 512],
                                start=True, stop=(r < 0))
                            if r >= 0:
                                # fold causal mask in: qk += I.T @ (-3e38
                                # staircase), only the 128-wide diagonal window
                                nc.tensor.matmul(
                                    qk[:, q2 * 512 + q0:q2 * 512 + q0 + 128],
                                    ident[:],
                                    mask[:, r * 512 + q0:r * 512 + q0 + 128],
                                    start=False, stop=True)
                        es = esp.tile([128, 1024], bf16, name="es")
                        if q0s[1] > 0:
                            for q2 in range(2):
                                q0 = q0s[q2]
                                nc.scalar.activation(
                                    es[:, q2 * 512 + q0:(q2 + 1) * 512],
                                    qk[:, q2 * 512 + q0:(q2 + 1) * 512], Exp)
                        else:
                            nc.scalar.activation(es[:], qk[:], Exp)
                        return es, q0s

                    def emit_av(tb, es, q0s):
                        for q2 in range(2):
                            tt = 2 * tb + q2
                            q0 = q0s[q2]
                            esl = es[:, q2 * 512 + q0:(q2 + 1) * 512]
                            nc.tensor.matmul(
                                av[:, q0:512], vN[:, tt * 128:(tt + 1) * 128],
                                esl, start=(tt == 0), stop=(tt == ntt - 1))
                            nc.tensor.matmul(
                                bps[:, q0:512], onem[:], esl,
                                start=(tt == 0), stop=(tt == ntt - 1))

                    prev = None
                    for tb in range(ntt // 2):
                        cur = (tb, *emit_qk(tb))
                        if prev is not None:
                            emit_av(*prev)
                        if ilv:
                            drain_piece(1)
                        prev = cur
                    emit_av(*prev)
                    recb = recp.tile([128, 512], f32, name="rec")
                    nc.vector.reciprocal(recb[:], bps[:])
                    nc.vector.tensor_tensor(attnT[sc][:, hs:hs + 512], av[:],
                                            recb[:], mult)

                if coll != "noout":
                    rs_in = [dramp.tile([rsrows, D], bf16, name=f"rs_in{b}",
                                        bufs=2) for b in range(rsc)]

                for sc in range(SC):
                    qTc = qkv_chunk(sc)
                    attnT[sc] = atp.tile([128, G * 512], bf16, name="attnT")
                    # prefetch next x chunk; wo after first chunk's x
                    if sc < SC - 1:
                        load_x(sc + 1)
                    if sc == 0 and not noattn:
                        for j in range(4):
                            dma(wo[:, j * 4096:(j + 1) * 4096],
                                wo_d[:, j * 4096:(j + 1) * 4096])
                    if noattn:
                        continue
                    for h in range(G):
                        attn_head(sc, h, qTc)
                        if sc >= 1:
                            queue_piece(sc - 1, 4 * (sc - 1) + h)
                            if not ilv:
                                drain_piece()
                    drain_piece()
                if noattn:
                    # consume qT/attnT rings so Tile sees them written+read
                    for sc in range(SC):
                        nc.vector.tensor_copy(attnT[sc][:, 0:512],
                                              kT[:, 0:512])
                    continue
                for st in range(12, 16):
                    queue_piece(3, st)
                drain_piece()
    nc.compile()
    return nc


def _prep_inputs(x, Wqkv, bqkv, Wo, bo):
    """Host-side shard prep. Returns in_maps for the 8 cores."""
    x0T = np.ascontiguousarray(np.asarray(x, F32)[0].T)          # [D, S]
    xt_t = np.ascontiguousarray(
        x0T.reshape(MC, 128, SC, 512).transpose(1, 2, 0, 3)
    ).reshape(128, SC * MC * 512).astype(BF16)

    Wqkv = np.asarray(Wqkv, F32)
    Wo = np.asarray(Wo, F32)
    bqkv = np.asarray(bqkv, F32)

    # rope tables (transposed [HD, S]); 1/sqrt(HD) split sqrt-wise onto q & k
    inv_freq = 1.0 / (BASE ** (np.arange(0, HD, 2, dtype=np.float64) / HD))
    t = np.arange(S, dtype=np.float64)
    freqs = np.outer(t, inv_freq)                                # [S, 64]
    emb = np.concatenate([freqs, freqs], axis=1)                 # [S, HD]
    s4 = SCALE ** 0.5
    cosT = (np.cos(emb).T * s4).astype(F32)                      # [HD, S]
    sinT = (np.sin(emb).T * s4).astype(F32)
    sin_signed = np.concatenate([-sinT[:64], sinT[64:]], axis=0)
    cos_h = np.ascontiguousarray(cosT).astype(BF16)
    sin_h = np.ascontiguousarray(sin_signed).astype(BF16)

    # additive causal masks for the 4 diagonal t-tiles: 0 where visible,
    # -3e38 where masked (added into the QK PSUM via an identity matmul)
    p = np.arange(128)[:, None]
    f = np.arange(512)[None, :]
    mask = np.stack([np.where(128 * r + p <= f, 0.0, -3e38).astype(F32)
                     for r in range(4)],
                    axis=1).reshape(128, 4 * 512).astype(BF16)

    ident = np.eye(128, dtype=np.float32).astype(BF16)
    onem = np.ones((128, 128), BF16)

    in_maps = []
    for g in range(N_CORES):
        # block-major: blk in (q0..q3, k, v), each [mc, 128p, 128cols]
        cols = [Wqkv[:, 512 * g + 128 * j:512 * g + 128 * (j + 1)]
                for j in range(G)]
        cols.append(Wqkv[:, D + 128 * g:D + 128 * (g + 1)])
        cols.append(Wqkv[:, D + KV + 128 * g:D + KV + 128 * (g + 1)])
        wqkv_g = np.ascontiguousarray(
            np.stack([c.reshape(MC, 128, 128) for c in cols], axis=0)
            .transpose(2, 0, 1, 3)   # [128p, blk, mc, 128]
        ).reshape(128, 6 * MC * 128).astype(BF16)
        wo_g = np.ascontiguousarray(
            Wo[512 * g:512 * (g + 1), :].reshape(G, 128, 8, 512)
            .transpose(1, 0, 2, 3)).reshape(128, G * 8 * 512).astype(BF16)
        bqw_g = np.concatenate([
            bqkv[512 * g:512 * (g + 1)],
            bqkv[D + 128 * g:D + 128 * (g + 1)],
            bqkv[D + KV + 128 * g:D + KV + 128 * (g + 1)],
        ]).reshape(1, 6 * 128).astype(BF16)
        in_maps.append({
            "xt": xt_t, "wqkv": wqkv_g, "wo": wo_g,
            "cos": cos_h, "sin": sin_h,
            "bqw": bqw_g, "ones": np.ones((1, 512), BF16), "mask": mask,
            "ident": ident, "onem": onem,
        })
    return in_maps


def kernel(x, Wqkv, bqkv, Wo, bo):
    hasbias = bool(np.any(np.asarray(bqkv)))
    key = ("nc", hasbias)
    if key not in _CACHE:
        _CACHE[key] = _build(reps=1, hasbias=hasbias)
    nc = _CACHE[key]
    in_maps = _prep_inputs(x, Wqkv, bqkv, Wo, bo)
    res = run_bass_kernel_spmd(nc, in_maps, core_ids=list(range(N_CORES)))
    # reassemble: core g's out block b (RSOUT rows) = final rows
    # [RSROWS*b + RSOUT*g : +RSOUT]
    out = np.zeros((S, D), F32)
    for g in range(N_CORES):
        og = np.asarray(res.results[g]["out"], F32)   # [SSLICE, D]
        for b in range(RSC):
            out[RSROWS * b + RSOUT * g:
                RSROWS * b + RSOUT * (g + 1)] = og[b * RSOUT:(b + 1) * RSOUT]
    out = out + np.asarray(bo, F32)[None, :]
    return out[None].astype(F32)
